# revision 4
# baseline (speedup 1.0000x reference)
"""Trainium2 Bass kernel for nn_LowpassDetector.

Computes: power = re^2 + im^2, 5-tap FIR (b), order-4 IIR recurrence (a)
along time, for signal [2, T=16384, B=2048] -> y [T, B].

Strategy: the FIR+IIR cascade is LTI with all poles at radius <= 0.758,
so the combined impulse response h decays below 1e-15 within 128 taps.
The whole filter is therefore exactly (to fp32) a block-Toeplitz matmul:
  y_blk[b] = T0 @ x_blk[b] + T1 @ x_blk[b-1]     (b >= 1)
  y_blk[0] = L0 @ x_blk[0]
where L0 is the exact 128x128 operator of the reference recurrence
(including its nonstandard "first 5 samples pass through" initial
condition), built on the host in float64 by running the reference on
basis vectors. Channels (2048) are sharded 256 per core across 8 cores;
time blocks of 128 map to the TensorEngine contraction dim.

v4 (this version) — DMA-roofline focused:
- Host hands each core a block-transposed, re/im-interleaved input
  [NSB, 128, 2*SBW*C] so each superbatch input is ONE 4 MB DMA whose
  descriptors are 32 KB fully-contiguous runs per partition (v3 moved
  the same bytes as ~50k separate 1 KB packets at ~334 GB/s).
- Output is written as fp16 ([NSB, 128, SBW*C]) and upcast to fp32 on
  the host: halves write traffic (50.3 -> 41.9 MB/core total), adds
  ~5e-4 max-rel rounding against a 2e-2 budget.
- Input DMAs issue on nc.sync (qSPDynamicHW), output DMAs + weight
  loads on nc.scalar (qActDynamicHW) so the two directions live on
  separate HWDGE queues and neither's semaphore waits stall the other.
- Matmuls run in fp16 (weights W~fp16, y = Wh@x); the only visible
  error is the single fp16 rounding of x (~2^-12) and of y (~2^-11).
"""

import sys
from contextlib import ExitStack

import numpy as np

for _p in ("/opt/trn_rl_repo",):
    if _p not in sys.path:
        sys.path.insert(0, _p)

import concourse.bass as bass  # noqa: E402
import concourse.tile as tile  # noqa: E402
from concourse import bacc, mybir  # noqa: E402
from concourse.bass_utils import run_bass_kernel_spmd  # noqa: E402

T, B, NCORES = 16384, 2048, 8
BL = 128                # time-block size (= PE contraction dim)
NB = T // BL            # 128 time blocks
C = B // NCORES         # 256 channels per core
SBW = 8                 # time blocks per superbatch
NSB = NB // SBW         # superbatches
F32 = mybir.dt.float32
F16 = mybir.dt.float16

TRACE = False           # set by test harness for NTFF profiling
LAST_RESULTS = None     # BassKernelResults of the last run (for profiling)

_program_cache = {}


def _reference_operator(bb, aa, n):
    """Exact linear operator of the reference filter on n samples (float64).

    Columns are responses to basis vectors; replicates the reference
    semantics: xf = zero-padded cross-correlation with b, first 5 outputs
    pass through, recurrence y[t] = xf[t] - sum_j a_j y[t-j] from t=5.
    """
    x = np.eye(n)
    xp = np.concatenate([np.zeros((4, n)), x], 0)
    xf = sum(bb[k] * xp[k:k + n] for k in range(5))
    y = xf.copy()
    at = aa[:4]
    for t in range(5, n):
        y[t] = xf[t] - (at[0] * y[t - 4] + at[1] * y[t - 3]
                        + at[2] * y[t - 2] + at[3] * y[t - 1])
    return y


def _build_mats(b32, a32):
    """Returns dict of fp16 stationary operands (transposed for lhsT)."""
    bb = np.asarray(b32, np.float64)
    aa = np.asarray(a32, np.float64)
    M = _reference_operator(bb, aa, 3 * BL)
    L0 = M[0:BL, 0:BL]
    T0 = M[2 * BL:3 * BL, 2 * BL:3 * BL]
    T1 = M[2 * BL:3 * BL, BL:2 * BL]
    # truncation + init-transient leakage must be below fp32 noise
    leak = np.abs(M[2 * BL:3 * BL, 0:BL]).max()
    dev = max(np.abs(M[BL:2 * BL, BL:2 * BL] - T0).max(),
              np.abs(M[BL:2 * BL, 0:BL] - T1).max())
    assert leak < 1e-9 and dev < 1e-9, (leak, dev)

    out = {}
    for name, W in (("l0h", L0), ("t0h", T0), ("t1h", T1)):
        WT = np.ascontiguousarray(W.T)          # matmul wants lhsT = W.T
        out[name] = np.ascontiguousarray(WT.astype(np.float16))
    return out


def _build_program():
    nc = bacc.Bacc("TRN2", target_bir_lowering=False, debug=False)
    M = SBW * C
    sig = nc.dram_tensor("sig", [NSB, BL, 2 * M], F32,
                         kind="ExternalInput").ap()
    wd = {n: nc.dram_tensor(n, [BL, BL], F16, kind="ExternalInput").ap()
          for n in ("l0h", "t0h", "t1h")}
    yd = nc.dram_tensor("y", [NSB, BL, M], F16, kind="ExternalOutput").ap()

    with tile.TileContext(nc) as tc, ExitStack() as ctx:
        wpool = ctx.enter_context(tc.tile_pool(name="w", bufs=1))
        w = {}
        for n, d in wd.items():
            w[n] = wpool.tile([BL, BL], F16, tag=n, name="w_" + n)
            nc.scalar.dma_start(w[n][:], d)     # off the input (sync) queue

        iopool = ctx.enter_context(tc.tile_pool(name="io", bufs=6))
        hpool = ctx.enter_context(tc.tile_pool(name="h", bufs=4))
        ypool = ctx.enter_context(tc.tile_pool(name="y", bufs=4))
        pspool = ctx.enter_context(tc.tile_pool(name="ps", bufs=4,
                                                space="PSUM"))

        def mm(ps_ap, wt, rhs_ap, start=False, stop=False):
            nc.tensor.matmul(ps_ap, w[wt][:], rhs_ap, start=start, stop=stop)

        prev_xh = None
        for s in range(NSB):
            x = iopool.tile([BL, 2 * M], F32, tag="x")
            nc.sync.dma_start(x[:], sig[s])     # one 4 MB, 32 KB/partition

            nc.scalar.activation(x[:], x[:],
                                 mybir.ActivationFunctionType.Square)
            # power, rounded once to fp16 by the add itself; col 0:C is a
            # margin holding the previous superbatch's last block (for the
            # cross-block T1 term).
            xh = hpool.tile([BL, C + M], F16, tag="xh")
            nc.vector.tensor_add(xh[:, C:], x[:, :M], x[:, M:])
            if s > 0:
                nc.vector.tensor_copy(xh[:, 0:C], prev_xh[:, M:])

            ysb = ypool.tile([BL, M], F16, tag="ysb")
            for q in range(SBW // 4):        # 2-bank psum per 2 pairs
                ps = pspool.tile([BL, 4 * C], F32, tag="ps")
                if s == 0 and q == 0:
                    # block 0 needs the exact-init operator L0 and no
                    # cross term; keep every matmul inside one psum bank.
                    mm(ps[:, 0:C], "l0h", xh[:, C:2 * C],
                       start=True, stop=True)
                    mm(ps[:, C:2 * C], "t0h", xh[:, 2 * C:3 * C], start=True)
                    mm(ps[:, C:2 * C], "t1h", xh[:, C:2 * C], stop=True)
                    mm(ps[:, 2 * C:4 * C], "t0h", xh[:, 3 * C:5 * C],
                       start=True)
                    mm(ps[:, 2 * C:4 * C], "t1h", xh[:, 2 * C:4 * C],
                       stop=True)
                else:
                    for i in range(2):
                        p = 2 * q + i            # pair = blocks 2p, 2p+1
                        pp = ps[:, i * 2 * C:(i + 1) * 2 * C]
                        cur = xh[:, C + p * 2 * C: C + (p + 1) * 2 * C]
                        sh = xh[:, p * 2 * C: (p + 1) * 2 * C]
                        mm(pp, "t0h", cur, start=True)
                        mm(pp, "t1h", sh, stop=True)

                dst = ysb[:, q * 4 * C:(q + 1) * 4 * C]
                if q % 2 == 0:
                    nc.scalar.activation(dst, ps[:],
                                         mybir.ActivationFunctionType.Copy)
                else:
                    nc.vector.tensor_copy(dst, ps[:])

            nc.scalar.dma_start(yd[s], ysb[:])  # 1 MB fp16, 8 KB/partition
            prev_xh = xh

    nc.compile()
    return nc


def kernel(signal, b, a):
    global LAST_RESULTS
    signal = np.asarray(signal, dtype=np.float32)
    assert signal.shape == (2, T, B), signal.shape

    wmats = _build_mats(np.asarray(b), np.asarray(a))

    if "prog" not in _program_cache:
        _program_cache["prog"] = _build_program()
    nc = _program_cache["prog"]

    # block-transpose + interleave: [2, T, B] -> [NSB, BL, 2, SBW, B]
    # (t = s*SBW*BL + blk*BL + p maps to [s, p, i, blk, :])
    sig_t = np.ascontiguousarray(
        signal.reshape(2, NSB, SBW, BL, B).transpose(1, 3, 0, 2, 4))

    in_maps = []
    for c in range(NCORES):
        sl = sig_t[:, :, :, :, c * C:(c + 1) * C]
        m = {"sig": np.ascontiguousarray(sl).reshape(NSB, BL, 2 * SBW * C)}
        m.update(wmats)
        in_maps.append(m)

    res = run_bass_kernel_spmd(nc, in_maps, core_ids=list(range(NCORES)),
                               trace=TRACE)
    LAST_RESULTS = res

    out = np.empty((T, B), np.float32)
    for c in range(NCORES):
        yc = res.results[c]["y"].reshape(NSB, BL, SBW, C)
        out[:, c * C:(c + 1) * C] = (
            yc.transpose(0, 2, 1, 3).reshape(T, C).astype(np.float32))
    return out


# revision 5
# speedup vs baseline: 1.1462x; 1.1462x over previous
"""Trainium2 Bass kernel for nn_LowpassDetector.

Computes: power = re^2 + im^2, 5-tap FIR (b), order-4 IIR recurrence (a)
along time, for signal [2, T=16384, B=2048] -> y [T, B].

Strategy: the FIR+IIR cascade is LTI with all poles at radius <= 0.758,
so the combined impulse response h decays below 1e-15 within 128 taps.
The whole filter is therefore exactly (to fp32) a block-Toeplitz matmul:
  y_blk[b] = T0 @ x_blk[b] + T1 @ x_blk[b-1]     (b >= 1)
  y_blk[0] = L0 @ x_blk[0]
where L0 is the exact 128x128 operator of the reference recurrence
(including its nonstandard "first 5 samples pass through" initial
condition), built on the host in float64 by running the reference on
basis vectors. Channels (2048) are sharded 256 per core across 8 cores;
time blocks of 128 map to the TensorEngine contraction dim.

v4 (this version) — DMA-roofline focused:
- Host hands each core a block-transposed, re/im-interleaved input
  [NSB, 128, 2*SBW*C] so each superbatch input is ONE 4 MB DMA whose
  descriptors are 32 KB fully-contiguous runs per partition (v3 moved
  the same bytes as ~50k separate 1 KB packets at ~334 GB/s).
- Output is written as fp16 ([NSB, 128, SBW*C]) and upcast to fp32 on
  the host: halves write traffic (50.3 -> 41.9 MB/core total), adds
  ~5e-4 max-rel rounding against a 2e-2 budget.
- Input DMAs issue on nc.sync (qSPDynamicHW), output DMAs + weight
  loads on nc.scalar (qActDynamicHW) so the two directions live on
  separate HWDGE queues and neither's semaphore waits stall the other.
- Matmuls run in fp16 (weights W~fp16, y = Wh@x); the only visible
  error is the single fp16 rounding of x (~2^-12) and of y (~2^-11).
"""

import sys
from contextlib import ExitStack

import numpy as np

for _p in ("/opt/trn_rl_repo",):
    if _p not in sys.path:
        sys.path.insert(0, _p)

import concourse.bass as bass  # noqa: E402
import concourse.tile as tile  # noqa: E402
from concourse import bacc, mybir  # noqa: E402
from concourse.bass_utils import run_bass_kernel_spmd  # noqa: E402

T, B, NCORES = 16384, 2048, 8
BL = 128                # time-block size (= PE contraction dim)
NB = T // BL            # 128 time blocks
C = B // NCORES         # 256 channels per core
SBW = 8                 # time blocks per superbatch
NSB = NB // SBW         # superbatches
F32 = mybir.dt.float32
F16 = mybir.dt.float16

TRACE = False           # set by test harness for NTFF profiling
LAST_RESULTS = None     # BassKernelResults of the last run (for profiling)

_program_cache = {}


def _reference_operator(bb, aa, n):
    """Exact linear operator of the reference filter on n samples (float64).

    Columns are responses to basis vectors; replicates the reference
    semantics: xf = zero-padded cross-correlation with b, first 5 outputs
    pass through, recurrence y[t] = xf[t] - sum_j a_j y[t-j] from t=5.
    """
    x = np.eye(n)
    xp = np.concatenate([np.zeros((4, n)), x], 0)
    xf = sum(bb[k] * xp[k:k + n] for k in range(5))
    y = xf.copy()
    at = aa[:4]
    for t in range(5, n):
        y[t] = xf[t] - (at[0] * y[t - 4] + at[1] * y[t - 3]
                        + at[2] * y[t - 2] + at[3] * y[t - 1])
    return y


def _build_mats(b32, a32):
    """Returns dict of fp16 stationary operands (transposed for lhsT)."""
    bb = np.asarray(b32, np.float64)
    aa = np.asarray(a32, np.float64)
    M = _reference_operator(bb, aa, 3 * BL)
    L0 = M[0:BL, 0:BL]
    T0 = M[2 * BL:3 * BL, 2 * BL:3 * BL]
    T1 = M[2 * BL:3 * BL, BL:2 * BL]
    # truncation + init-transient leakage must be below fp32 noise
    leak = np.abs(M[2 * BL:3 * BL, 0:BL]).max()
    dev = max(np.abs(M[BL:2 * BL, BL:2 * BL] - T0).max(),
              np.abs(M[BL:2 * BL, 0:BL] - T1).max())
    assert leak < 1e-9 and dev < 1e-9, (leak, dev)

    out = {}
    for name, W in (("l0h", L0), ("t0h", T0), ("t1h", T1)):
        WT = np.ascontiguousarray(W.T)          # matmul wants lhsT = W.T
        out[name] = np.ascontiguousarray(WT.astype(np.float16))
    return out


def _build_program():
    nc = bacc.Bacc("TRN2", target_bir_lowering=False, debug=False)
    M = SBW * C
    sig = nc.dram_tensor("sig", [NSB, BL, 2 * M], F32,
                         kind="ExternalInput").ap()
    wd = {n: nc.dram_tensor(n, [BL, BL], F16, kind="ExternalInput").ap()
          for n in ("l0h", "t0h", "t1h")}
    yd = nc.dram_tensor("y", [NSB, BL, M], F16, kind="ExternalOutput").ap()

    with tile.TileContext(nc) as tc, ExitStack() as ctx:
        wpool = ctx.enter_context(tc.tile_pool(name="w", bufs=1))
        w = {}
        for n, d in wd.items():
            w[n] = wpool.tile([BL, BL], F16, tag=n, name="w_" + n)
            nc.scalar.dma_start(w[n][:], d)     # off the input (sync) queue

        iopool = ctx.enter_context(tc.tile_pool(name="io", bufs=6))
        hpool = ctx.enter_context(tc.tile_pool(name="h", bufs=4))
        ypool = ctx.enter_context(tc.tile_pool(name="y", bufs=4))
        pspool = ctx.enter_context(tc.tile_pool(name="ps", bufs=4,
                                                space="PSUM"))

        def mm(ps_ap, wt, rhs_ap, start=False, stop=False):
            nc.tensor.matmul(ps_ap, w[wt][:], rhs_ap, start=start, stop=stop)

        hs = [None] * NSB

        def stage_a(s):
            """Input DMA (sync), square (scalar), power-add (vector)."""
            x = iopool.tile([BL, 2 * M], F32, tag="x")
            nc.sync.dma_start(x[:], sig[s])     # 2 MB, 16 KB/partition

            nc.scalar.activation(x[:], x[:],
                                 mybir.ActivationFunctionType.Square)
            # power, rounded once to fp16 by the add itself; col 0:C is a
            # margin holding the previous superbatch's last block (for the
            # cross-block T1 term).
            xh = hpool.tile([BL, C + M], F16, tag="xh")
            hs[s] = xh
            nc.vector.tensor_add(xh[:, C:], x[:, :M], x[:, M:])
            if s > 0:
                nc.vector.tensor_copy(xh[:, 0:C], hs[s - 1][:, M:])

        def stage_b(s):
            """Matmuls (PE), psum drains (scalar/vector), out DMA (gpsimd)."""
            xh = hs[s]
            ysb = ypool.tile([BL, M], F16, tag="ysb")
            for q in range(SBW // 4):        # 2-bank psum per 2 pairs
                ps = pspool.tile([BL, 4 * C], F32, tag="ps")
                if s == 0 and q == 0:
                    # block 0 needs the exact-init operator L0 and no
                    # cross term; keep every matmul inside one psum bank.
                    mm(ps[:, 0:C], "l0h", xh[:, C:2 * C],
                       start=True, stop=True)
                    mm(ps[:, C:2 * C], "t0h", xh[:, 2 * C:3 * C], start=True)
                    mm(ps[:, C:2 * C], "t1h", xh[:, C:2 * C], stop=True)
                    mm(ps[:, 2 * C:4 * C], "t0h", xh[:, 3 * C:5 * C],
                       start=True)
                    mm(ps[:, 2 * C:4 * C], "t1h", xh[:, 2 * C:4 * C],
                       stop=True)
                else:
                    for i in range(2):
                        p = 2 * q + i            # pair = blocks 2p, 2p+1
                        pp = ps[:, i * 2 * C:(i + 1) * 2 * C]
                        cur = xh[:, C + p * 2 * C: C + (p + 1) * 2 * C]
                        sh = xh[:, p * 2 * C: (p + 1) * 2 * C]
                        mm(pp, "t0h", cur, start=True)
                        mm(pp, "t1h", sh, stop=True)

                dst = ysb[:, q * 4 * C:(q + 1) * 4 * C]
                if q % 2 == 0:
                    nc.scalar.activation(dst, ps[:],
                                         mybir.ActivationFunctionType.Copy)
                else:
                    nc.vector.tensor_copy(dst, ps[:])

            nc.gpsimd.dma_start(yd[s], ysb[:])  # SWDGE: own queue/engine

        # software pipeline: each engine streams its own stage — stage-A
        # work of superbatch s issues before stage-B of s-1 so no engine's
        # in-order queue blocks its next-superbatch front-end work behind
        # a downstream dependency (drain/out-DMA) of the previous one.
        for s in range(NSB):
            stage_a(s)
            if s >= 1:
                stage_b(s - 1)
        stage_b(NSB - 1)

    nc.compile()
    return nc


def kernel(signal, b, a):
    global LAST_RESULTS
    signal = np.asarray(signal, dtype=np.float32)
    assert signal.shape == (2, T, B), signal.shape

    wmats = _build_mats(np.asarray(b), np.asarray(a))

    if "prog" not in _program_cache:
        _program_cache["prog"] = _build_program()
    nc = _program_cache["prog"]

    # block-transpose + interleave: [2, T, B] -> [NSB, BL, 2, SBW, B]
    # (t = s*SBW*BL + blk*BL + p maps to [s, p, i, blk, :])
    sig_t = np.ascontiguousarray(
        signal.reshape(2, NSB, SBW, BL, B).transpose(1, 3, 0, 2, 4))

    in_maps = []
    for c in range(NCORES):
        sl = sig_t[:, :, :, :, c * C:(c + 1) * C]
        m = {"sig": np.ascontiguousarray(sl).reshape(NSB, BL, 2 * SBW * C)}
        m.update(wmats)
        in_maps.append(m)

    res = run_bass_kernel_spmd(nc, in_maps, core_ids=list(range(NCORES)),
                               trace=TRACE)
    LAST_RESULTS = res

    out = np.empty((T, B), np.float32)
    for c in range(NCORES):
        yc = res.results[c]["y"].reshape(NSB, BL, SBW, C)
        out[:, c * C:(c + 1) * C] = (
            yc.transpose(0, 2, 1, 3).reshape(T, C).astype(np.float32))
    return out
